# revision 15
# baseline (speedup 1.0000x reference)
"""Trainium2 Bass kernel for CausalSelfAttention with retrieval prefix.

Reference computation (B=2, T=2048, C=1024, H=16, P=256, hd=64):
    q = rope(x @ Wq), k = rope(x_key @ Wkv[:, :C]), v = x_key @ Wkv[:, C:]
    att = softmax(concat([att_prefix, causal(q k^T / 8)], -1))
    y = (att_t @ v + att_p @ cache_v) @ Wo

Sharding: 8 cores = 2 batches x 4 head-groups (4 heads each).  Each core
computes its batch/heads fully including its slice of the output projection
(Wo row-parallel); host sums the 4 partial projections per batch.

v2 design notes (cost model: matmul cost = moving-cols * 0.4167ns):
  - S^T [keys, q] psum pair per (dg, kc): two heads' tiles adjacent so one
    exp activation covers both (amortizes Act's ~185ns/op overhead).
  - att@v flipped: PT [keys, 128q] is the matmul *stationary*, v [keys, 65]
    moving -> out y [128q, 4qc, 65] accumulates over kc in one psum bank.
    65 moving cols/chunk vs 512 in the naive orientation; the ones column
    yields the softmax denominator per-partition so normalization is a
    reciprocal + per-partition tensor_scalar_mul.
  - exp(att_prefix) precomputed on host (input-only data, Act is scarce).
  - RoPE in fp16 (DVE 2x mode) from a Pool-staged fp16 copy of the psum.
  - Diagonal S blocks N-restricted to q >= key chunk start; triangle mask
    applied post-exp via a constant [128,2,128] 0/1 tile.
  - y [q, feat] transposed back per 128-block on PE for the Wo projection.
"""

import os

import ml_dtypes
import numpy as np


def _enable_jax_cache():
    try:
        import jax
        cache_dir = os.environ.get("BASS_JAX_CACHE", "/root/.cache/jax_bass")
        os.makedirs(cache_dir, exist_ok=True)
        jax.config.update("jax_compilation_cache_dir", cache_dir)
        jax.config.update("jax_persistent_cache_min_compile_time_secs", 0.0)
        jax.config.update("jax_persistent_cache_min_entry_size_bytes", 0)
    except Exception:
        pass


_enable_jax_cache()

B, T, C, H, P = 2, 2048, 1024, 16, 256
HD = C // H  # 64
NCORES = 8
HPC = 4            # heads per core
QT = 512           # q tile
NTT = 512          # phase1 token tile

_cache = {}


def _build():
    import concourse.bacc as bacc
    import concourse.mybir as mybir
    import concourse.tile as tile

    F32 = mybir.dt.float32
    FP16 = mybir.dt.float16
    EXP = mybir.ActivationFunctionType.Exp

    nc = bacc.Bacc("TRN2")

    xt_d = nc.dram_tensor("xt", [C, T], FP16, kind="ExternalInput").ap()
    xkt_d = nc.dram_tensor("xkt", [C, T], FP16, kind="ExternalInput").ap()
    wq_d = nc.dram_tensor("wq", [C, 256], FP16, kind="ExternalInput").ap()
    wk_d = nc.dram_tensor("wk", [C, 256], FP16, kind="ExternalInput").ap()
    wv_d = nc.dram_tensor("wv", [C, 256], FP16, kind="ExternalInput").ap()
    wo_d = nc.dram_tensor("wo", [256, C], FP16, kind="ExternalInput").ap()
    pref_d = nc.dram_tensor("prefE", [128, 2, HPC, T], FP16, kind="ExternalInput").ap()
    cv_d = nc.dram_tensor("cvaug", [128, 2, HPC, 65], FP16, kind="ExternalInput").ap()
    vones_d = nc.dram_tensor("vones", [128, 16, HPC], FP16, kind="ExternalInput").ap()
    cos_d = nc.dram_tensor("cosS", [128, T], FP16, kind="ExternalInput").ap()
    sin_d = nc.dram_tensor("sinSh", [128, T], FP16, kind="ExternalInput").ap()
    mask_d = nc.dram_tensor("maskTri", [128, 2, 128], FP16, kind="ExternalInput").ap()
    id_d = nc.dram_tensor("ident", [128, 128], FP16, kind="ExternalInput").ap()
    oy_d = nc.dram_tensor("oy", [T, C], FP16, kind="ExternalOutput").ap()

    NST = T // NTT  # 4

    with tile.TileContext(nc) as tc:
        with (
            tc.tile_pool(name="pers", bufs=1) as pers,
            tc.tile_pool(name="cst", bufs=1) as cst,
            tc.tile_pool(name="p1x", bufs=2) as p1x,
            tc.tile_pool(name="stg", bufs=3) as stg,
            tc.tile_pool(name="rtp", bufs=2) as rtp,
            tc.tile_pool(name="ptp", bufs=32) as ptp,
            tc.tile_pool(name="ynp", bufs=2) as ynp,
            tc.tile_pool(name="rcpp", bufs=2) as rcpp,
            tc.tile_pool(name="osbp", bufs=3) as osbp,
            tc.tile_pool(name="ps512", bufs=2, space="PSUM") as ps512,
            tc.tile_pool(name="spsp", bufs=2, space="PSUM") as spsp,
            tc.tile_pool(name="ytp", bufs=2, space="PSUM") as ytp,
        ):
            qT = pers.tile([128, 2, T], FP16)
            kT = pers.tile([128, 2, T], FP16)
            vh = pers.tile([128, 16, HPC, 65], FP16)
            yT = pers.tile([128, 2, T], FP16)
            nc.sync.dma_start(out=vh[:, :, :, 64:65], in_=vones_d[:, :, :, None])

            wo = cst.tile([128, 2, C], FP16)
            nc.sync.dma_start(out=wo[:], in_=wo_d.rearrange("(a p) n -> p a n", p=128))
            maskTri = cst.tile([128, 2, 128], FP16)
            nc.sync.dma_start(out=maskTri[:], in_=mask_d[:])
            cvP = cst.tile([128, 2, HPC, 65], FP16)
            nc.sync.dma_start(out=cvP[:], in_=cv_d[:])
            prefP = cst.tile([128, 2, HPC, T], FP16)
            nc.sync.dma_start(out=prefP[:], in_=pref_d[:])
            ident = cst.tile([128, 128], FP16)
            nc.sync.dma_start(out=ident[:], in_=id_d[:])
            wq = cst.tile([128, 8, 256], FP16)
            wk = cst.tile([128, 8, 256], FP16)
            wv = cst.tile([128, 8, 256], FP16)
            for w_sb, w_dd in ((wq, wq_d), (wk, wk_d), (wv, wv_d)):
                nc.sync.dma_start(
                    out=w_sb[:], in_=w_dd.rearrange("(a p) m -> p a m", p=128)
                )
            cosS = cst.tile([128, T], FP16)
            sinSh = cst.tile([128, T], FP16)
            nc.sync.dma_start(out=cosS[:], in_=cos_d[:])
            nc.sync.dma_start(out=sinSh[:], in_=sin_d[:])

            def phase1(tt):
                t0 = tt * NTT
                tsl = slice(t0, t0 + NTT)
                xt = p1x.tile([128, 8, NTT], FP16, tag="xt", name=f"xt{tt}")
                xkt = p1x.tile([128, 8, NTT], FP16, tag="xkt", name=f"xkt{tt}")
                nc.sync.dma_start(
                    out=xt[:],
                    in_=xt_d.rearrange("(a p) t -> p a t", p=128)[:, :, tsl],
                )
                nc.sync.dma_start(
                    out=xkt[:],
                    in_=xkt_d.rearrange("(a p) t -> p a t", p=128)[:, :, tsl],
                )
                for dg in range(2):
                    dsl = slice(dg * 128, dg * 128 + 128)
                    for src, dst in ((xt, qT), (xkt, kT)):
                        ps = ps512.tile([128, NTT], F32, tag="ps")
                        for kc in range(8):
                            nc.tensor.matmul(
                                ps[:],
                                (wq if dst is qT else wk)[:, kc, dsl],
                                src[:, kc, :],
                                start=(kc == 0),
                                stop=(kc == 7),
                            )
                        qr = stg.tile([128, NTT], FP16, tag="qr")
                        nc.scalar.copy(qr[:], ps[:])
                        tmp = rtp.tile([128, NTT], FP16, tag="tmp")
                        ssw = rtp.tile([128, NTT], FP16, tag="ssw")
                        nc.vector.tensor_mul(tmp[:], qr[:], cosS[:, tsl])
                        for hb in (0, 64):
                            a = slice(hb, hb + 32)
                            bsl = slice(hb + 32, hb + 64)
                            nc.vector.tensor_mul(
                                ssw[a, :], qr[bsl, :], sinSh[bsl, tsl]
                            )
                            nc.vector.tensor_mul(
                                ssw[bsl, :], qr[a, :], sinSh[a, tsl]
                            )
                        nc.vector.tensor_add(dst[:, dg, tsl], tmp[:], ssw[:])
                    # v projection: two token-chunks per psum tile
                for vp in range(2):
                    pv = ps512.tile([128, NTT], F32, tag="ps", name=f"pv{tt}_{vp}")
                    pvv = pv.rearrange("p (a b) -> p a b", a=2)
                    for i in range(2):
                        lo = (vp * 2 + i) * 128
                        for kc in range(8):
                            nc.tensor.matmul(
                                pvv[:, i, :],
                                xkt[:, kc, lo:lo + 128],
                                wv[:, kc, :],
                                start=(kc == 0),
                                stop=(kc == 7),
                            )
                    ttg = (t0 + vp * 256) // 128
                    nc.vector.tensor_copy(
                        vh[:, ttg:ttg + 2, :, 0:64],
                        pvv.rearrange("p a (h d) -> p a h d", h=HPC),
                    )

            def phase2(qt):
                q0 = qt * QT
                nkc = (q0 + QT) // 128
                yN = ynp.tile([128, 4, 256], FP16, tag="yn", name=f"yn{qt}")
                yns.append(yN)
                # S + exp for both head-pairs first (PE streams S while Act
                # exps one chunk behind), then att@v consumes the kept PT
                # tiles as closed per-bank accumulation groups (HW PSUM: a
                # start=True while another group in the same bank is open
                # clobbers it).
                PTs = {}

                def s_chunk(dg, kc):
                    r = kc * 128 - q0
                    c0 = max(r, 0)
                    ksl = slice(kc * 128, kc * 128 + 128)
                    sps = spsp.tile([128, 2, QT], F32, tag="sps")
                    for hh in range(2):
                        base = hh * 64
                        hsl = slice(base, base + 64)
                        nc.tensor.matmul(
                            sps[:, hh, c0:],
                            kT[hsl, dg, ksl],
                            qT[hsl, dg, q0 + c0:q0 + QT],
                            start=True,
                            stop=True,
                            tile_position=(base, 0),
                        )
                    PT = ptp.tile([128, 2, QT], FP16, tag="pt")
                    nc.scalar.activation(
                        PT[:, :, c0:], sps[:, :, c0:], EXP, scale=0.125
                    )
                    if r >= 0:
                        nc.gpsimd.tensor_mul(
                            PT[:, :, r:r + 128], PT[:, :, r:r + 128], maskTri[:]
                        )
                    PTs[(dg, kc)] = PT

                def av_group(dg, hh, qc):
                    hl = 2 * dg + hh
                    qsl = slice(q0 + qc * 128, q0 + qc * 128 + 128)
                    y = ytp.tile([128, 65], F32, tag="yt", name=f"y{qt}_{dg}{hh}{qc}")
                    for pc in range(2):
                        nc.tensor.matmul(
                            y[:],
                            prefP[:, pc, hl, qsl],
                            cvP[:, pc, hl, :],
                            start=(pc == 0),
                            stop=False,
                        )
                    for kc in range(4 * qt + qc + 1):
                        nc.tensor.matmul(
                            y[:],
                            PTs[(dg, kc)][:, hh, qc * 128:qc * 128 + 128],
                            vh[:, kc, hl, :],
                            start=False,
                            stop=(kc == 4 * qt + qc),
                        )
                    rcp = rcpp.tile([128, 1], F32, tag="rcp")
                    nc.vector.reciprocal(rcp[:], y[:, 64:65])
                    nc.vector.tensor_scalar_mul(
                        yN[:, qc, hl * 64:hl * 64 + 64], y[:, 0:64], rcp[:]
                    )

                for kc in range(nkc):
                    s_chunk(0, kc)
                # dg0's att@v groups fill PE while Act exps dg1's S chunks
                groups0 = [(0, hh, qc) for hh in range(2) for qc in range(4)]
                gi = 0
                for kc in range(nkc):
                    s_chunk(1, kc)
                    want = (kc + 1) * 8 // nkc if kc > 0 else 0
                    while gi < want:
                        av_group(*groups0[gi])
                        gi += 1
                while gi < 8:
                    av_group(*groups0[gi])
                    gi += 1
                for hh in range(2):
                    for qc in range(4):
                        av_group(1, hh, qc)

            def phase2_tail(qt):
                q0 = qt * QT
                yN = yns.pop(0)
                for qc in range(4):
                    for dg in range(2):
                        tps = ytp.tile(
                            [128, 128], FP16, tag="yt", name=f"tp{qt}_{qc}{dg}"
                        )
                        nc.tensor.transpose(
                            tps[:], yN[:, qc, dg * 128:dg * 128 + 128], ident[:]
                        )
                        nc.vector.tensor_copy(
                            yT[:, dg, q0 + qc * 128:q0 + qc * 128 + 128], tps[:]
                        )

            def phase3(qt):
                q0 = qt * QT
                for tt in range(QT // 128):
                    tsl = slice(q0 + tt * 128, q0 + tt * 128 + 128)
                    for ncol in range(2):
                        nsl = slice(ncol * 512, ncol * 512 + 512)
                        ops = ps512.tile([128, 512], F32, tag="ps", name=f"op{qt}_{tt}{ncol}")
                        for dg in range(2):
                            nc.tensor.matmul(
                                ops[:],
                                yT[:, dg, tsl],
                                wo[:, dg, nsl],
                                start=(dg == 0),
                                stop=(dg == 1),
                            )
                        osb = osbp.tile([128, 512], FP16, tag="osb")
                        nc.vector.tensor_copy(osb[:], ops[:])
                        nc.sync.dma_start(out=oy_d[tsl, nsl], in_=osb[:])

            # main loop: phase1(st) fills the PE while rope catches up; the
            # previous tile's transposes+projection slot into the gap before
            # phase2's S pipeline (which is Act-bound) takes over.
            yns = []
            for st in range(NST):
                phase1(st)
                if st > 0:
                    phase2_tail(st - 1)
                    phase3(st - 1)
                phase2(st)
            phase2_tail(NST - 1)
            phase3(NST - 1)

    nc.compile()
    return nc


def _host_prep(inputs):
    x = np.asarray(inputs["x"], dtype=np.float32)
    x_key = np.asarray(inputs["x_key"], dtype=np.float32)
    Wq = np.asarray(inputs["Wq"], dtype=np.float32)
    Wkv = np.asarray(inputs["Wkv"], dtype=np.float32)
    Wo = np.asarray(inputs["Wo"], dtype=np.float32)
    pref = np.asarray(inputs["att_prefix"], dtype=np.float32)
    cache_v = np.asarray(inputs["cache_v"], dtype=np.float32)
    start_index = int(np.asarray(inputs["start_index"]))

    # rope tables: row p -> inv_freq[p % 32]; sinSh carries the rotate-half
    # sign (negative for the first half of each 64-dim head block).
    half = HD // 2
    pos = (start_index + np.arange(T)).astype(np.float64)
    inv_freq = (10000.0 ** (-np.arange(half, dtype=np.float64) / half))
    ang = pos[None, :] * inv_freq[np.arange(128) % half][:, None]  # [128, T]
    cosS = np.cos(ang).astype(np.float16)
    # sinSh row p multiplies qr row p (inputs must share base partition);
    # the product lands in the partner row, so the rotate-half sign (-1 for
    # the x2 term) sits on the rows p%64 >= 32 that feed the x1 outputs.
    sgn = np.where((np.arange(128) % 64) < 32, 1.0, -1.0)
    sinSh = (np.sin(ang) * sgn[:, None]).astype(np.float16)

    # triangle keep-mask for diagonal key chunks: keep iff q_local >= k_local
    tri = (np.arange(128)[None, :] >= np.arange(128)[:, None]).astype(np.float16)
    maskTri = np.repeat(tri[:, None, :], 2, axis=1).copy()

    Wk_full = Wkv[:, :C]
    Wv_full = Wkv[:, C:]
    prefE = np.exp(pref).astype(np.float16)  # [B, H, T, P]

    in_maps = []
    for c in range(NCORES):
        b, g = divmod(c, HPC)
        cols = slice(256 * g, 256 * (g + 1))
        heads = slice(HPC * g, HPC * (g + 1))
        cvaug = np.zeros((128, 2, HPC, 65), dtype=np.float16)
        cv = cache_v[b, heads]  # [4, 256, 64]
        cvaug[:, :, :, :64] = cv.reshape(HPC, 2, 128, 64).transpose(2, 1, 0, 3).astype(np.float16)
        cvaug[:, :, :, 64] = 1.0
        # prefE slice -> [128 kp, 2 pc, HPC, T]
        pe = prefE[b, heads]  # [HPC, T, P]
        pe = pe.reshape(HPC, T, 2, 128).transpose(3, 2, 0, 1).copy()
        in_maps.append({
            "xt": np.ascontiguousarray(x[b].T).astype(np.float16),
            "xkt": np.ascontiguousarray(x_key[b].T).astype(np.float16),
            "wq": np.ascontiguousarray(Wq[:, cols]).astype(np.float16),
            "wk": np.ascontiguousarray(Wk_full[:, cols]).astype(np.float16),
            "wv": np.ascontiguousarray(Wv_full[:, cols]).astype(np.float16),
            "wo": np.ascontiguousarray(Wo[cols, :]).astype(np.float16),
            "prefE": pe,
            "cvaug": cvaug,
            "vones": np.ones((128, 16, HPC), dtype=np.float16),
            "cosS": cosS,
            "sinSh": sinSh,
            "maskTri": maskTri,
            "ident": np.eye(128, dtype=np.float16),
        })
    return in_maps


def kernel(**inputs) -> np.ndarray:
    from concourse.bass_utils import run_bass_kernel_spmd

    if "nc" not in _cache:
        _cache["nc"] = _build()
    nc = _cache["nc"]

    in_maps = _host_prep(inputs)
    res = run_bass_kernel_spmd(nc, in_maps, core_ids=list(range(NCORES)))
    outs = [res.results[c]["oy"].astype(np.float32) for c in range(NCORES)]
    y = np.zeros((B, T, C), dtype=np.float32)
    for c in range(NCORES):
        b = c // HPC
        y[b] += outs[c]
    return y


# revision 17
# speedup vs baseline: 1.0891x; 1.0891x over previous
"""Trainium2 Bass kernel for CausalSelfAttention with retrieval prefix.

Reference computation (B=2, T=2048, C=1024, H=16, P=256, hd=64):
    q = rope(x @ Wq), k = rope(x_key @ Wkv[:, :C]), v = x_key @ Wkv[:, C:]
    att = softmax(concat([att_prefix, causal(q k^T / 8)], -1))
    y = (att_t @ v + att_p @ cache_v) @ Wo

Sharding: 8 cores = 2 batches x 4 head-groups (4 heads each).  Each core
computes its batch/heads fully including its slice of the output projection
(Wo row-parallel); host sums the 4 partial projections per batch.

v2 design notes (cost model: matmul cost = moving-cols * 0.4167ns):
  - S^T [keys, q] psum pair per (dg, kc): two heads' tiles adjacent so one
    exp activation covers both (amortizes Act's ~185ns/op overhead).
  - att@v flipped: PT [keys, 128q] is the matmul *stationary*, v [keys, 65]
    moving -> out y [128q, 4qc, 65] accumulates over kc in one psum bank.
    65 moving cols/chunk vs 512 in the naive orientation; the ones column
    yields the softmax denominator per-partition so normalization is a
    reciprocal + per-partition tensor_scalar_mul.
  - exp(att_prefix) precomputed on host (input-only data, Act is scarce).
  - RoPE in fp16 (DVE 2x mode) from a Pool-staged fp16 copy of the psum.
  - Diagonal S blocks N-restricted to q >= key chunk start; triangle mask
    applied post-exp via a constant [128,2,128] 0/1 tile.
  - y [q, feat] transposed back per 128-block on PE for the Wo projection.
"""

import os

import ml_dtypes
import numpy as np


def _enable_jax_cache():
    try:
        import jax
        cache_dir = os.environ.get("BASS_JAX_CACHE", "/root/.cache/jax_bass")
        os.makedirs(cache_dir, exist_ok=True)
        jax.config.update("jax_compilation_cache_dir", cache_dir)
        jax.config.update("jax_persistent_cache_min_compile_time_secs", 0.0)
        jax.config.update("jax_persistent_cache_min_entry_size_bytes", 0)
    except Exception:
        pass


_enable_jax_cache()

B, T, C, H, P = 2, 2048, 1024, 16, 256
HD = C // H  # 64
NCORES = 8
HPC = 4            # heads per core
QT = 512           # q tile
NTT = 512          # phase1 token tile

_cache = {}


def _build():
    import concourse.bacc as bacc
    import concourse.mybir as mybir
    import concourse.tile as tile

    F32 = mybir.dt.float32
    FP16 = mybir.dt.float16
    EXP = mybir.ActivationFunctionType.Exp

    nc = bacc.Bacc("TRN2")

    xt_d = nc.dram_tensor("xt", [C, T], FP16, kind="ExternalInput").ap()
    xkt_d = nc.dram_tensor("xkt", [C, T], FP16, kind="ExternalInput").ap()
    wq_d = nc.dram_tensor("wq", [C, 256], FP16, kind="ExternalInput").ap()
    wk_d = nc.dram_tensor("wk", [C, 256], FP16, kind="ExternalInput").ap()
    wv_d = nc.dram_tensor("wv", [C, 256], FP16, kind="ExternalInput").ap()
    wo_d = nc.dram_tensor("wo", [256, C], FP16, kind="ExternalInput").ap()
    pref_d = nc.dram_tensor("prefE", [128, 2, HPC, T], FP16, kind="ExternalInput").ap()
    cv_d = nc.dram_tensor("cvaug", [128, 2, HPC, 65], FP16, kind="ExternalInput").ap()
    vones_d = nc.dram_tensor("vones", [128, 16, HPC], FP16, kind="ExternalInput").ap()
    cos_d = nc.dram_tensor("cosS", [128, T], FP16, kind="ExternalInput").ap()
    sin_d = nc.dram_tensor("sinSh", [128, T], FP16, kind="ExternalInput").ap()
    mask_d = nc.dram_tensor("maskTri", [128, 2, 128], FP16, kind="ExternalInput").ap()
    id_d = nc.dram_tensor("ident", [128, 128], FP16, kind="ExternalInput").ap()
    oy_d = nc.dram_tensor("oy", [T, C], FP16, kind="ExternalOutput").ap()

    NST = T // NTT  # 4

    with tile.TileContext(nc) as tc:
        with (
            tc.tile_pool(name="pers", bufs=1) as pers,
            tc.tile_pool(name="cst", bufs=1) as cst,
            tc.tile_pool(name="p1x", bufs=2) as p1x,
            tc.tile_pool(name="stg", bufs=3) as stg,
            tc.tile_pool(name="rtp", bufs=2) as rtp,
            tc.tile_pool(name="ptp", bufs=32) as ptp,
            tc.tile_pool(name="ynp", bufs=2) as ynp,
            tc.tile_pool(name="rcpp", bufs=2) as rcpp,
            tc.tile_pool(name="osbp", bufs=3) as osbp,
            tc.tile_pool(name="ps512", bufs=2, space="PSUM") as ps512,
            tc.tile_pool(name="spsp", bufs=2, space="PSUM") as spsp,
            tc.tile_pool(name="ytp", bufs=2, space="PSUM") as ytp,
        ):
            qT = pers.tile([128, 2, T], FP16)
            kT = pers.tile([128, 2, T], FP16)
            vh = pers.tile([128, 16, HPC, 65], FP16)
            yT = pers.tile([128, 2, T], FP16)

            # DMAs in first-use order: the proj weights and the first x tiles
            # gate phase1(0); the 4MB prefix table isn't read until the first
            # att@v group (~40us in), so it must not sit ahead of them in the
            # DMA queue.
            wq = cst.tile([128, 8, 256], FP16)
            wk = cst.tile([128, 8, 256], FP16)
            nc.sync.dma_start(
                out=wq[:], in_=wq_d.rearrange("(a p) m -> p a m", p=128)
            )
            nc.sync.dma_start(
                out=wk[:], in_=wk_d.rearrange("(a p) m -> p a m", p=128)
            )

            def make_x(tt):
                t0 = tt * NTT
                tsl = slice(t0, t0 + NTT)
                xt = p1x.tile([128, 8, NTT], FP16, tag="xt", name=f"xt{tt}")
                xkt = p1x.tile([128, 8, NTT], FP16, tag="xkt", name=f"xkt{tt}")
                nc.sync.dma_start(
                    out=xt[:],
                    in_=xt_d.rearrange("(a p) t -> p a t", p=128)[:, :, tsl],
                )
                nc.sync.dma_start(
                    out=xkt[:],
                    in_=xkt_d.rearrange("(a p) t -> p a t", p=128)[:, :, tsl],
                )
                return xt, xkt

            x0 = make_x(0)

            cosS = cst.tile([128, T], FP16)
            sinSh = cst.tile([128, T], FP16)
            nc.sync.dma_start(out=cosS[:], in_=cos_d[:])
            nc.sync.dma_start(out=sinSh[:], in_=sin_d[:])
            wv = cst.tile([128, 8, 256], FP16)
            nc.sync.dma_start(
                out=wv[:], in_=wv_d.rearrange("(a p) m -> p a m", p=128)
            )
            nc.sync.dma_start(out=vh[:, :, :, 64:65], in_=vones_d[:, :, :, None])
            maskTri = cst.tile([128, 2, 128], FP16)
            nc.sync.dma_start(out=maskTri[:], in_=mask_d[:])
            cvP = cst.tile([128, 2, HPC, 65], FP16)
            nc.sync.dma_start(out=cvP[:], in_=cv_d[:])
            ident = cst.tile([128, 128], FP16)
            nc.sync.dma_start(out=ident[:], in_=id_d[:])
            prefP = cst.tile([128, 2, HPC, T], FP16)
            nc.sync.dma_start(out=prefP[:], in_=pref_d[:])
            wo = cst.tile([128, 2, C], FP16)
            nc.sync.dma_start(out=wo[:], in_=wo_d.rearrange("(a p) n -> p a n", p=128))

            def phase1(tt, pre=None):
                t0 = tt * NTT
                tsl = slice(t0, t0 + NTT)
                xt, xkt = pre if pre is not None else make_x(tt)
                for dg in range(2):
                    dsl = slice(dg * 128, dg * 128 + 128)
                    for src, dst in ((xt, qT), (xkt, kT)):
                        ps = ps512.tile([128, NTT], F32, tag="ps")
                        for kc in range(8):
                            nc.tensor.matmul(
                                ps[:],
                                (wq if dst is qT else wk)[:, kc, dsl],
                                src[:, kc, :],
                                start=(kc == 0),
                                stop=(kc == 7),
                            )
                        qr = stg.tile([128, NTT], FP16, tag="qr")
                        nc.scalar.copy(qr[:], ps[:])
                        tmp = rtp.tile([128, NTT], FP16, tag="tmp")
                        ssw = rtp.tile([128, NTT], FP16, tag="ssw")
                        nc.vector.tensor_mul(tmp[:], qr[:], cosS[:, tsl])
                        for hb in (0, 64):
                            a = slice(hb, hb + 32)
                            bsl = slice(hb + 32, hb + 64)
                            nc.vector.tensor_mul(
                                ssw[a, :], qr[bsl, :], sinSh[bsl, tsl]
                            )
                            nc.vector.tensor_mul(
                                ssw[bsl, :], qr[a, :], sinSh[a, tsl]
                            )
                        nc.vector.tensor_add(dst[:, dg, tsl], tmp[:], ssw[:])
                    # v projection: two token-chunks per psum tile
                for vp in range(2):
                    pv = ps512.tile([128, NTT], F32, tag="ps", name=f"pv{tt}_{vp}")
                    pvv = pv.rearrange("p (a b) -> p a b", a=2)
                    for i in range(2):
                        lo = (vp * 2 + i) * 128
                        for kc in range(8):
                            nc.tensor.matmul(
                                pvv[:, i, :],
                                xkt[:, kc, lo:lo + 128],
                                wv[:, kc, :],
                                start=(kc == 0),
                                stop=(kc == 7),
                            )
                    ttg = (t0 + vp * 256) // 128
                    nc.vector.tensor_copy(
                        vh[:, ttg:ttg + 2, :, 0:64],
                        pvv.rearrange("p a (h d) -> p a h d", h=HPC),
                    )

            def phase2(qt):
                q0 = qt * QT
                nkc = (q0 + QT) // 128
                yN = ynp.tile([128, 4, 256], FP16, tag="yn", name=f"yn{qt}")
                yns.append(yN)
                # S + exp for both head-pairs first (PE streams S while Act
                # exps one chunk behind), then att@v consumes the kept PT
                # tiles as closed per-bank accumulation groups (HW PSUM: a
                # start=True while another group in the same bank is open
                # clobbers it).
                PTs = {}

                def s_chunk(dg, kc):
                    r = kc * 128 - q0
                    c0 = max(r, 0)
                    ksl = slice(kc * 128, kc * 128 + 128)
                    sps = spsp.tile([128, 2, QT], F32, tag="sps")
                    for hh in range(2):
                        base = hh * 64
                        hsl = slice(base, base + 64)
                        nc.tensor.matmul(
                            sps[:, hh, c0:],
                            kT[hsl, dg, ksl],
                            qT[hsl, dg, q0 + c0:q0 + QT],
                            start=True,
                            stop=True,
                            tile_position=(base, 0),
                        )
                    PT = ptp.tile([128, 2, QT], FP16, tag="pt")
                    nc.scalar.activation(
                        PT[:, :, c0:], sps[:, :, c0:], EXP, scale=0.125
                    )
                    if r >= 0:
                        nc.gpsimd.tensor_mul(
                            PT[:, :, r:r + 128], PT[:, :, r:r + 128], maskTri[:]
                        )
                    PTs[(dg, kc)] = PT

                def av_group(dg, hh, qc):
                    hl = 2 * dg + hh
                    qsl = slice(q0 + qc * 128, q0 + qc * 128 + 128)
                    y = ytp.tile([128, 65], F32, tag="yt", name=f"y{qt}_{dg}{hh}{qc}")
                    for pc in range(2):
                        nc.tensor.matmul(
                            y[:],
                            prefP[:, pc, hl, qsl],
                            cvP[:, pc, hl, :],
                            start=(pc == 0),
                            stop=False,
                        )
                    for kc in range(4 * qt + qc + 1):
                        nc.tensor.matmul(
                            y[:],
                            PTs[(dg, kc)][:, hh, qc * 128:qc * 128 + 128],
                            vh[:, kc, hl, :],
                            start=False,
                            stop=(kc == 4 * qt + qc),
                        )
                    rcp = rcpp.tile([128, 1], F32, tag="rcp")
                    nc.vector.reciprocal(rcp[:], y[:, 64:65])
                    nc.vector.tensor_scalar_mul(
                        yN[:, qc, hl * 64:hl * 64 + 64], y[:, 0:64], rcp[:]
                    )

                for kc in range(nkc):
                    s_chunk(0, kc)
                # dg0's att@v groups fill PE while Act exps dg1's S chunks
                groups0 = [(0, hh, qc) for hh in range(2) for qc in range(4)]
                gi = 0
                for kc in range(nkc):
                    s_chunk(1, kc)
                    want = (kc + 1) * 8 // nkc if kc > 0 else 0
                    while gi < want:
                        av_group(*groups0[gi])
                        gi += 1
                while gi < 8:
                    av_group(*groups0[gi])
                    gi += 1
                for hh in range(2):
                    for qc in range(4):
                        av_group(1, hh, qc)

            def phase2_tail(qt):
                q0 = qt * QT
                yN = yns.pop(0)
                for qc in range(4):
                    for dg in range(2):
                        tps = ytp.tile(
                            [128, 128], FP16, tag="yt", name=f"tp{qt}_{qc}{dg}"
                        )
                        nc.tensor.transpose(
                            tps[:], yN[:, qc, dg * 128:dg * 128 + 128], ident[:]
                        )
                        nc.vector.tensor_copy(
                            yT[:, dg, q0 + qc * 128:q0 + qc * 128 + 128], tps[:]
                        )

            def phase3(qt):
                q0 = qt * QT
                for tt in range(QT // 128):
                    tsl = slice(q0 + tt * 128, q0 + tt * 128 + 128)
                    for ncol in range(2):
                        nsl = slice(ncol * 512, ncol * 512 + 512)
                        ops = ps512.tile([128, 512], F32, tag="ps", name=f"op{qt}_{tt}{ncol}")
                        for dg in range(2):
                            nc.tensor.matmul(
                                ops[:],
                                yT[:, dg, tsl],
                                wo[:, dg, nsl],
                                start=(dg == 0),
                                stop=(dg == 1),
                            )
                        osb = osbp.tile([128, 512], FP16, tag="osb")
                        nc.vector.tensor_copy(osb[:], ops[:])
                        nc.sync.dma_start(out=oy_d[tsl, nsl], in_=osb[:])

            # main loop: phase1(st) fills the PE while rope catches up; the
            # previous tile's transposes+projection slot into the gap before
            # phase2's S pipeline (which is Act-bound) takes over.
            yns = []
            for st in range(NST):
                phase1(st, pre=(x0 if st == 0 else None))
                if st > 0:
                    phase2_tail(st - 1)
                    phase3(st - 1)
                phase2(st)
            phase2_tail(NST - 1)
            phase3(NST - 1)

    nc.compile()
    return nc


def _host_prep(inputs):
    x = np.asarray(inputs["x"], dtype=np.float32)
    x_key = np.asarray(inputs["x_key"], dtype=np.float32)
    Wq = np.asarray(inputs["Wq"], dtype=np.float32)
    Wkv = np.asarray(inputs["Wkv"], dtype=np.float32)
    Wo = np.asarray(inputs["Wo"], dtype=np.float32)
    pref = np.asarray(inputs["att_prefix"], dtype=np.float32)
    cache_v = np.asarray(inputs["cache_v"], dtype=np.float32)
    start_index = int(np.asarray(inputs["start_index"]))

    # rope tables: row p -> inv_freq[p % 32]; sinSh carries the rotate-half
    # sign (negative for the first half of each 64-dim head block).
    half = HD // 2
    pos = (start_index + np.arange(T)).astype(np.float64)
    inv_freq = (10000.0 ** (-np.arange(half, dtype=np.float64) / half))
    ang = pos[None, :] * inv_freq[np.arange(128) % half][:, None]  # [128, T]
    cosS = np.cos(ang).astype(np.float16)
    # sinSh row p multiplies qr row p (inputs must share base partition);
    # the product lands in the partner row, so the rotate-half sign (-1 for
    # the x2 term) sits on the rows p%64 >= 32 that feed the x1 outputs.
    sgn = np.where((np.arange(128) % 64) < 32, 1.0, -1.0)
    sinSh = (np.sin(ang) * sgn[:, None]).astype(np.float16)

    # triangle keep-mask for diagonal key chunks: keep iff q_local >= k_local
    tri = (np.arange(128)[None, :] >= np.arange(128)[:, None]).astype(np.float16)
    maskTri = np.repeat(tri[:, None, :], 2, axis=1).copy()

    Wk_full = Wkv[:, :C]
    Wv_full = Wkv[:, C:]
    prefE = np.exp(pref).astype(np.float16)  # [B, H, T, P]

    in_maps = []
    for c in range(NCORES):
        b, g = divmod(c, HPC)
        cols = slice(256 * g, 256 * (g + 1))
        heads = slice(HPC * g, HPC * (g + 1))
        cvaug = np.zeros((128, 2, HPC, 65), dtype=np.float16)
        cv = cache_v[b, heads]  # [4, 256, 64]
        cvaug[:, :, :, :64] = cv.reshape(HPC, 2, 128, 64).transpose(2, 1, 0, 3).astype(np.float16)
        cvaug[:, :, :, 64] = 1.0
        # prefE slice -> [128 kp, 2 pc, HPC, T]
        pe = prefE[b, heads]  # [HPC, T, P]
        pe = pe.reshape(HPC, T, 2, 128).transpose(3, 2, 0, 1).copy()
        in_maps.append({
            "xt": np.ascontiguousarray(x[b].T).astype(np.float16),
            "xkt": np.ascontiguousarray(x_key[b].T).astype(np.float16),
            "wq": np.ascontiguousarray(Wq[:, cols]).astype(np.float16),
            "wk": np.ascontiguousarray(Wk_full[:, cols]).astype(np.float16),
            "wv": np.ascontiguousarray(Wv_full[:, cols]).astype(np.float16),
            "wo": np.ascontiguousarray(Wo[cols, :]).astype(np.float16),
            "prefE": pe,
            "cvaug": cvaug,
            "vones": np.ones((128, 16, HPC), dtype=np.float16),
            "cosS": cosS,
            "sinSh": sinSh,
            "maskTri": maskTri,
            "ident": np.eye(128, dtype=np.float16),
        })
    return in_maps


def kernel(**inputs) -> np.ndarray:
    from concourse.bass_utils import run_bass_kernel_spmd

    if "nc" not in _cache:
        _cache["nc"] = _build()
    nc = _cache["nc"]

    in_maps = _host_prep(inputs)
    res = run_bass_kernel_spmd(nc, in_maps, core_ids=list(range(NCORES)))
    outs = [res.results[c]["oy"].astype(np.float32) for c in range(NCORES)]
    y = np.zeros((B, T, C), dtype=np.float32)
    for c in range(NCORES):
        b = c // HPC
        y[b] += outs[c]
    return y
